# revision 65
# baseline (speedup 1.0000x reference)
"""CliffordLinear (Cl(3,0)) Trainium2 kernel — Karatsuba/bf16 edition.

Math: Cl(3,0) ~= M2(C) via the Pauli representation phi.  The reference
out[b,o] = sum_i W[o,i] * X[b,i] (Clifford product) maps to
OutM[b,o] = sum_i phi(W[o,i]) @ phi(X[b,i]).  Per output column c of the
2x2 matrix, with Xc = Ac + i*Bc ([B x 512] over (i,m)) and
Wm = R + i*I ([512 x 512] over [(i,m) x (o,r)]):

    Out_c = (Ac@R - Bc@I) + i(Ac@I + Bc@R)

computed with the 3-multiplication Karatsuba form
    M1 = Ac@R, M2 = Bc@I, M3 = (Ac+Bc)@(R+I)
    Re = M1 - M2,  Im = M3 - M1 - M2
which is 24 real MACs per (b,o,i) vs 32 for the 4-mult form.  All matmul
operands are bf16 (same PE rate as fp32r, half the HBM traffic); M-plane
recombination runs on ScalarE (PSUM->SBUF bf16 evict) + DVE (2x bf16
mode).  The blade <-> Pauli basis changes on both the input and output
side are host-side (free): the device ships raw Re/Im planes and the
host applies the inverse Pauli butterfly + bias.

Startup is DMA-latency bound: boot1=[x0A|R0] carries exactly what the
first matmul needs, boot2=[x0B|I0] the second; the remaining weights
stream as [R_k|I_k] chunks matching bt0's k-major matmul order, and
SW = R+I is computed on-device (4 DVE adds) to keep it off the wire.
M1/M2 accumulate in a 2-bank PSUM tile (psA) and M3 in a separate
1-bank tile (psB) so ScalarE and DVE can evict them concurrently (the
tile framework serializes multi-engine readers of one PSUM tile).
The last b-tile ships raw M-planes (host recombines them in fp32):
c0 full-width, then c1 split 384/128 columns so the final exposed
chain is three tiny copies plus one small DMA.

Sharding: data-parallel over batch (1024 rows/core); weights replicated.
Per-core HBM traffic: 4.2 MB x + 1.05 MB w in, 4.3 MB out (bf16).
"""

import sys

sys.path.insert(0, "/opt/trn_rl_repo")

import numpy as np
import ml_dtypes

import concourse.bass as bass  # noqa: F401  (registers lowerings)
import concourse.mybir as mybir
import concourse.tile as tile
from concourse import bacc
from concourse.bass_utils import run_bass_kernel_spmd

N_CORES = 8
B, CIN, COUT, NB = 8192, 256, 256, 8
BS = B // N_CORES          # 1024 batch rows per core
HK = 512                   # contraction rows (i,m) per complex half
KT = HK // 128             # 4 k-tiles
BT = BS // 128             # 8 b-tiles
OUTW = 2048                # out cols: [c0re|c0im| c1...] each 512 (o*2+r)

BF16 = ml_dtypes.bfloat16

_cached = {}


def _build_nc():
    f32 = mybir.dt.float32
    bf16 = mybir.dt.bfloat16
    nc = bacc.Bacc("TRN2", target_bir_lowering=False, debug=False,
                   num_devices=N_CORES)
    # boot1: [x0A | R_k0] — everything the very first matmul needs.
    boot1 = nc.dram_tensor("boot1", [128, 1024], bf16, kind="ExternalInput")
    # boot2: [x0B | I_k0]; SW = R+I is computed on-device (DVE, once).
    boot2 = nc.dram_tensor("boot2", [128, 1024], bf16, kind="ExternalInput")
    # wk123: [R_k | I_k] for k=1..3
    wk123 = nc.dram_tensor("wk123", [3, 128, 1024], bf16, kind="ExternalInput")
    # x layout [bt, p, col]: col = c*1024 + plane*512 + kt*128 + b
    # (plane 0 = Ac = Re, plane 1 = Bc = Im); kappa = kt*128 + p.
    xt = nc.dram_tensor("xt", [BT, 128, 2048], bf16, kind="ExternalInput")
    out = nc.dram_tensor("out", [BS, OUTW], bf16, kind="ExternalOutput")
    # Last b-tile ships raw M-planes (host recombines in fp32):
    # c0: m1|m2|m3 (512 each); c1 h0: 3x384; c1 h1: 3x128.
    tailout = nc.dram_tensor("tailout", [128, 3072], bf16,
                             kind="ExternalOutput")

    with tile.TileContext(nc) as tc:
        with tc.tile_pool(name="wpool", bufs=1) as wpool, \
             tc.tile_pool(name="xpool", bufs=3) as xpool, \
             tc.tile_pool(name="spool", bufs=2) as spool, \
             tc.tile_pool(name="mpool", bufs=2) as mpool, \
             tc.tile_pool(name="opool", bufs=3) as opool, \
             tc.tile_pool(name="pspool", bufs=2, space="PSUM") as pspool:
            # PE warmup: a single tiny matmul starts the 3us p-state ramp
            # clock (pe_busy_start is set at first PE activity, no reset).
            warm_in = wpool.tile([128, 256], bf16, tag="warm_in")
            nc.gpsimd.memset(warm_in[:], 0.0)
            warm_ps = pspool.tile([128, 512], f32, tag="psB", bufs=3)
            nc.tensor.matmul(warm_ps[:, 0:128], warm_in[:, :128],
                             warm_in[:, 128:256], start=True, stop=True)

            boot1_t = wpool.tile([128, 1024], bf16, tag="boot1")
            nc.sync.dma_start(boot1_t[:], boot1[:, :])
            boot2_t = wpool.tile([128, 1024], bf16, tag="boot2")
            nc.sync.dma_start(boot2_t[:], boot2[:, :])
            x0cd = xpool.tile([128, 1024], bf16, tag="x0cd", bufs=1)
            nc.sync.dma_start(x0cd[:], xt[0][:, 1024:2048])
            wk_t = [None] * KT
            x1_t = xpool.tile([128, 2048], bf16, tag="x")
            for k in range(1, KT):
                wk_t[k] = wpool.tile([128, 1024], bf16, tag=f"wk{k}",
                                     name=f"wk{k}")
                nc.sync.dma_start(wk_t[k][:], wk123[k - 1])
                if k == 1:
                    nc.sync.dma_start(x1_t[:, 0:1024], xt[1][:, 0:1024])
            nc.sync.dma_start(x1_t[:, 1024:2048], xt[1][:, 1024:2048])
            # SW = R + I on DVE (once per k-chunk, off the DMA wire)
            sw_t = []
            for k in range(KT):
                sw = wpool.tile([128, 512], bf16, tag=f"sw{k}", name=f"sw{k}")
                if k == 0:
                    nc.vector.tensor_add(sw[:], boot1_t[:, 512:1024],
                                         boot2_t[:, 512:1024])
                else:
                    nc.vector.tensor_add(sw[:], wk_t[k][:, 0:512],
                                         wk_t[k][:, 512:1024])
                sw_t.append(sw)

            def W(p, k):
                """rhs chunk for plane p (0=R, 1=I, 2=SW), k-tile k."""
                if p == 2:
                    return sw_t[k][:]
                if k == 0:
                    return (boot1_t[:, 512:1024], boot2_t[:, 512:1024])[p]
                return wk_t[k][:, p * 512:(p + 1) * 512]

            for bt in range(BT):
                if bt == 0:
                    planes = [boot1_t[:, 0:512], boot2_t[:, 0:512],
                              x0cd[:, 0:512], x0cd[:, 512:1024]]
                else:
                    if bt == 1:
                        x_s = x1_t
                    else:
                        x_s = xpool.tile([128, 2048], bf16, tag="x")
                        nc.sync.dma_start(x_s[:], xt[bt])
                    planes = [x_s[:, q * 512:(q + 1) * 512] for q in range(4)]
                last = bt == BT - 1
                lhs = []
                for c in range(2):
                    a_p, b_p = planes[2 * c], planes[2 * c + 1]
                    s_p = spool.tile([128, 512], bf16, tag=f"s{c}",
                                     name=f"s{c}")
                    nc.vector.tensor_add(s_p[:], a_p, b_p)
                    lhs.append((a_p, b_p, s_p[:]))
                def _mm(psa, psb, c, pi, k, cols=None, _bt=bt):
                    """One accumulation matmul: M1/M2 -> psA, M3 -> psB."""
                    if cols is None:
                        dst = psb[:, 0:512] if pi == 2 else \
                            psa[:, pi * 512:(pi + 1) * 512]
                        w = W(pi, k)
                    else:
                        cw = cols.stop - cols.start
                        dst = psb[:, 0:cw] if pi == 2 else \
                            psa[:, pi * 512:pi * 512 + cw]
                        w = W(pi, k)[:, cols]
                    nc.tensor.matmul(dst, lhs[c][pi][:, k * 128:(k + 1) * 128],
                                     w, start=(k == 0), stop=(k == KT - 1))

                def _evict(psa, psb, c, _bt=bt):
                    """Recombine M1/M2/M3 -> [re|im] bf16 and DMA out.
                    Act evicts psA while DVE evicts psB (separate tiles so
                    the readers run concurrently)."""
                    m12 = mpool.tile([128, 1024], bf16, tag="m12", name="m12")
                    nc.scalar.copy(m12[:], psa[:])
                    m3s = mpool.tile([128, 512], bf16, tag="m3s", name="m3s")
                    nc.vector.tensor_copy(m3s[:], psb[:])
                    stage = opool.tile([128, 1024], bf16, tag="stage",
                                       name="stage")
                    t = opool.tile([128, 512], bf16, tag="t", name="t")
                    m1, m2 = m12[:, 0:512], m12[:, 512:1024]
                    nc.vector.tensor_sub(stage[:, 0:512], m1, m2)
                    nc.vector.tensor_sub(t[:], m3s[:], m1)
                    nc.vector.tensor_sub(stage[:, 512:1024], t[:], m2)
                    nc.scalar.dma_start(
                        out[_bt * 128:(_bt + 1) * 128,
                            c * 1024:(c + 1) * 1024], stage[:])

                def _ps_pair(nm):
                    psa = pspool.tile([128, 1024], f32, tag="psA",
                                      name=f"psA{nm}")
                    psb = pspool.tile([128, 512], f32, tag="psB",
                                      name=f"psB{nm}", bufs=3)
                    return psa, psb

                if not last:
                    for c in range(2):
                        psa, psb = _ps_pair(f"{bt}c{c}")
                        # bt0: k-major (matches DMA arrival); later: plane-major
                        order = ([(pi, k) for k in range(KT) for pi in range(3)]
                                 if bt == 0 else
                                 [(pi, k) for pi in range(3) for k in range(KT)])
                        for pi, k in order:
                            _mm(psa, psb, c, pi, k)
                        _evict(psa, psb, c)
                    continue
                # ---- Last b-tile: ship raw M-planes via tailout ----
                # c0: full-width matmuls; evict m12 on Act, m3 on DVE.
                psa, psb = _ps_pair("7c0")
                for pi in range(3):
                    for k in range(KT):
                        _mm(psa, psb, 0, pi, k)
                tm12 = opool.tile([128, 1024], bf16, tag="tm12")
                nc.scalar.copy(tm12[:], psa[:])
                tm3 = opool.tile([128, 512], bf16, tag="tm3")
                nc.vector.tensor_copy(tm3[:], psb[:])
                nc.scalar.dma_start(tailout[:, 0:1024], tm12[:])
                nc.sync.dma_start(tailout[:, 1024:1536], tm3[:])
                # c1 h0 (384 cols): copies split DVE/Act/DVE.
                psa, psb = _ps_pair("7h0")
                for pi in range(3):
                    for k in range(KT):
                        _mm(psa, psb, 1, pi, k, cols=slice(0, 384))
                th = opool.tile([128, 1152], bf16, tag="th0")
                nc.vector.tensor_copy(th[:, 0:384], psa[:, 0:384])
                nc.scalar.copy(th[:, 384:768], psa[:, 512:896])
                nc.vector.tensor_copy(th[:, 768:1152], psb[:, 0:384])
                nc.sync.dma_start(tailout[:, 1536:2688], th[:])
                # c1 h1 (last 128 cols): tiny DVE copies per plane-stop,
                # one small final DMA on sync.
                psa, psb = _ps_pair("7h1")
                for pi in range(3):
                    for k in range(KT):
                        _mm(psa, psb, 1, pi, k, cols=slice(384, 512))
                th1 = opool.tile([128, 384], bf16, tag="th1")
                nc.vector.tensor_copy(th1[:, 0:128], psa[:, 0:128])
                nc.vector.tensor_copy(th1[:, 128:256], psa[:, 512:640])
                nc.vector.tensor_copy(th1[:, 256:384], psb[:, 0:128])
                nc.sync.dma_start(tailout[:, 2688:3072], th1[:])
    nc.finalize()
    return nc


def _pauli_cols(v):
    """v[..., 8] -> (A0, B0, A1, B1): Re/Im lhs planes for column c of
    phi(v), each [..., 2] with the 2x2-matrix row index last."""
    v0, v1, v2, v3, v4, v5, v6, v7 = (v[..., a] for a in range(8))
    A0 = np.stack([v0 + v4, v1 + v5], axis=-1)   # Re(P00), Re(P10)
    B0 = np.stack([v3 + v7, v6 + v2], axis=-1)   # Im(P00), Im(P10)
    A1 = np.stack([v1 - v5, v0 - v4], axis=-1)   # Re(P01), Re(P11)
    B1 = np.stack([v6 - v2, v7 - v3], axis=-1)   # Im(P01), Im(P11)
    return A0, B0, A1, B1


def _prep_w(weight):
    """weight [COUT, CIN, 8] -> R, I, SW=R+I [512, 512] f32 planes of
    phi(W)[r,m] indexed [(i,m), (o,r)], with the 0.5 inverse factor folded."""
    w = weight.astype(np.float32)
    A0, B0, A1, B1 = _pauli_cols(w)  # [o, i, entry-row] for columns m=0,1
    R = np.empty((CIN, 2, COUT, 2), np.float32)   # [(i,m),(o,r)]
    I = np.empty_like(R)
    for m, (re_c, im_c) in ((0, (A0, B0)), (1, (A1, B1))):
        for r in range(2):
            R[:, m, :, r] = 0.5 * re_c[:, :, r].T
            I[:, m, :, r] = 0.5 * im_c[:, :, r].T
    R = R.reshape(HK, HK)
    I = I.reshape(HK, HK)
    return R, I, R + I


def _prep_x(x):
    """x [B, CIN, 8] -> [N_CORES, BT, 128, 2048] bf16 in the kernel's
    [bt, p, (c, plane, kt, b)] layout."""
    xf = x.astype(np.float32)
    A0, B0, A1, B1 = _pauli_cols(xf)             # [B, CIN, m]
    out = np.empty((B, 4, CIN * 2), np.float32)
    for q, arr in enumerate((A0, B0, A1, B1)):
        out[:, q, :] = arr.reshape(B, CIN * 2)
    # [B, (c,plane), kappa] -> [core, bt, p, c*1024+plane*512+kt*128+b]
    a = out.reshape(N_CORES, BT, 128, 4, KT, 128)  # [core, bt, b, cp, kt, p]
    a = a.transpose(0, 1, 5, 3, 4, 2)              # [core, bt, p, cp, kt, b]
    return np.ascontiguousarray(
        a.reshape(N_CORES, BT, 128, 2048)).astype(BF16)


def kernel(x, weight, bias, cayley):
    assert x.shape == (B, CIN, NB) and weight.shape == (COUT, CIN, NB)
    if "nc" not in _cached:
        _cached["nc"] = _build_nc()
    nc = _cached["nc"]

    xt = _prep_x(np.asarray(x))
    R, I, _SW = _prep_w(np.asarray(weight))
    wk123 = np.stack([np.concatenate(
        [R[k * 128:(k + 1) * 128], I[k * 128:(k + 1) * 128]], axis=1)
        for k in range(1, KT)], axis=0).astype(BF16)
    r0 = R[0:128].astype(BF16)
    i0 = I[0:128].astype(BF16)
    in_maps = []
    for c in range(N_CORES):
        b1 = np.concatenate([xt[c, 0, :, 0:512], r0], axis=1)
        b2 = np.concatenate([xt[c, 0, :, 512:1024], i0], axis=1)
        in_maps.append({"boot1": np.ascontiguousarray(b1),
                        "boot2": np.ascontiguousarray(b2),
                        "wk123": wk123, "xt": xt[c]})
    res = run_bass_kernel_spmd(nc, in_maps, core_ids=list(range(N_CORES)))
    devs = []
    for c in range(N_CORES):
        d = np.asarray(res.results[c]["out"]).astype(np.float32)
        # bt==BT-1 rows ship raw M-planes via tailout; recombine in fp32.
        tl = np.asarray(res.results[c]["tailout"]).astype(np.float32)
        m1_0 = tl[:, 0:512]
        m2_0 = tl[:, 512:1024]
        m3_0 = tl[:, 1024:1536]
        h0, h1 = tl[:, 1536:2688], tl[:, 2688:3072]  # 3x384 | 3x128
        m1_1 = np.concatenate([h0[:, 0:384], h1[:, 0:128]], axis=1)
        m2_1 = np.concatenate([h0[:, 384:768], h1[:, 128:256]], axis=1)
        m3_1 = np.concatenate([h0[:, 768:1152], h1[:, 256:384]], axis=1)
        d[BS - 128:, 0:512] = m1_0 - m2_0
        d[BS - 128:, 512:1024] = m3_0 - m1_0 - m2_0
        d[BS - 128:, 1024:1536] = m1_1 - m2_1
        d[BS - 128:, 1536:2048] = m3_1 - m1_1 - m2_1
        devs.append(d)
    dev = np.concatenate(devs, axis=0)
    re0, im0 = dev[:, 0:512], dev[:, 512:1024]
    re1, im1 = dev[:, 1024:1536], dev[:, 1536:2048]
    o = np.empty((B, COUT, NB), np.float32)
    o[..., 0] = (re0[:, 0::2] + re1[:, 1::2]).reshape(B, COUT)
    o[..., 4] = (re0[:, 0::2] - re1[:, 1::2]).reshape(B, COUT)
    o[..., 7] = (im0[:, 0::2] + im1[:, 1::2]).reshape(B, COUT)
    o[..., 3] = (im0[:, 0::2] - im1[:, 1::2]).reshape(B, COUT)
    o[..., 1] = (re0[:, 1::2] + re1[:, 0::2]).reshape(B, COUT)
    o[..., 5] = (re0[:, 1::2] - re1[:, 0::2]).reshape(B, COUT)
    o[..., 6] = (im0[:, 1::2] + im1[:, 0::2]).reshape(B, COUT)
    o[..., 2] = (im0[:, 1::2] - im1[:, 0::2]).reshape(B, COUT)
    o += np.asarray(bias, np.float32)[None]
    return o.astype(np.float32)
